# revision 2
# baseline (speedup 1.0000x reference)
"""Causal MHA (batch=4, seq=2048, d_model=768, 12 heads of 64) on 8 TRN2 cores.

v2: qc-major schedule. The baseline ran all attention head-major and only
then the output projection + 16 output DMAs, fully serialized (~210us of
the ~470us total). Here each 512-query chunk (qc) finishes all 6 heads,
then immediately runs its output projection and one batched output DMA,
overlapping out-proj/DMA of chunk qc with attention of chunk qc+1.

Sharding: core c handles batch c//2 and heads (c%2)*6 .. +6. Host sums the
two half-partials per batch and adds biases (b_V folds through softmax).

Device math (per core), fp16 matmuls with fp32 PSUM accumulation:
  QT[hd, q] = (Wq/8).T @ x.T + bq/8   KT[hd, q] = Wk.T @ x.T + bk
  V[k, hd]  = x @ Wv  ([V|1] augmented)
  ST[k, q]  = KT.T @ QT per head, causal strips; PT = exp(ST) (no max-shift)
  z_aug = [V|1].T @ PT -> rows 0..63 = z, row 64 = sum
  zT = z[0:64] / z[64]  (recip + partition broadcast; pair-packed [128, q])
  out[q, m] = sum_j ztp_j.T @ Wo_pair_j
"""
import contextlib
import numpy as np

import concourse.bass as bass
import concourse.mybir as mybir
import concourse.tile as tile
from concourse import bacc
from concourse.bass_utils import run_bass_kernel_spmd
from concourse.masks import make_identity, make_lower_triangular

BATCH, SEQ, DM, NH, DH = 4, 2048, 768, 12, 64
H = 6                 # heads per core
HD = H * DH           # 384
MC = DM // 128        # 6 m-chunks
NKT = SEQ // 128      # 16 k-tiles
NQC = SEQ // 512      # 4 q-chunks
F16 = mybir.dt.float16
F32 = mybir.dt.float32

_BUILD_CACHE = {}


def build(reps: int = 1, upto: str = "all"):
    key = (reps, upto)
    if key in _BUILD_CACHE:
        return _BUILD_CACHE[key]
    nc = bacc.Bacc("TRN2", target_bir_lowering=False, debug=False)
    xt_d = nc.dram_tensor("xt", [DM, SEQ], F16, kind="ExternalInput")
    wq_d = nc.dram_tensor("wq", [128, MC, HD], F16, kind="ExternalInput")
    wk_d = nc.dram_tensor("wk", [128, MC, HD], F16, kind="ExternalInput")
    wv_d = nc.dram_tensor("wv", [128, MC, HD], F16, kind="ExternalInput")
    wo_d = nc.dram_tensor("wo", [3, 128, DM], F16, kind="ExternalInput")
    bq_d = nc.dram_tensor("bq", [128, HD // 128], F32, kind="ExternalInput")
    bk_d = nc.dram_tensor("bk", [128, HD // 128], F32, kind="ExternalInput")
    o_d = nc.dram_tensor("out", [SEQ, DM], F16, kind="ExternalOutput")

    with tile.TileContext(nc) as tc:
        def body(_iv=None):
            with contextlib.ExitStack() as ctx:
                consts = ctx.enter_context(tc.tile_pool(name="consts", bufs=1))
                persist = ctx.enter_context(tc.tile_pool(name="persist", bufs=1))

                # ---- load inputs (already fp16 from host) ----
                # order matters: the load queue is FIFO, so biases and the
                # Q/K weights go first; the first projection matmul can then
                # start as soon as x-chunk 0 lands.
                bq_s = consts.tile([128, HD // 128], F32)
                nc.sync.dma_start(out=bq_s, in_=bq_d.ap())
                bk_s = consts.tile([128, HD // 128], F32)
                nc.sync.dma_start(out=bk_s, in_=bk_d.ap())
                w16 = {}
                for name, d in [("wq", wq_d), ("wk", wk_d)]:
                    wt = persist.tile([128, MC, HD], F16, name=f"{name}16")
                    nc.sync.dma_start(out=wt, in_=d.ap())
                    w16[name] = wt
                xt16 = []
                for c in range(MC):
                    xc = persist.tile([128, SEQ], F16, name=f"xt16_{c}")
                    nc.sync.dma_start(out=xc,
                                      in_=xt_d.ap()[c * 128:(c + 1) * 128, :])
                    xt16.append(xc)
                for name, d in [("wv", wv_d)]:
                    wt = persist.tile([128, MC, HD], F16, name=f"{name}16")
                    nc.sync.dma_start(out=wt, in_=d.ap())
                    w16[name] = wt
                wo16 = []
                for j in range(3):
                    wot = persist.tile([128, DM], F16, name=f"wo16_{j}")
                    nc.sync.dma_start(out=wot, in_=wo_d.ap()[j])
                    wo16.append(wot)

                ident = consts.tile([128, 128], F16)
                make_identity(nc, ident)
                mneg = consts.tile([128, 128], F16)
                make_lower_triangular(nc, mneg, val=-30000.0, diag=False)
                ones_r = consts.tile([1, 64], F16)
                nc.vector.memset(ones_r, 1.0)

                if upto == "load":
                    return

                if upto.startswith("mmprobe"):
                    # pure PE throughput probe: back-to-back accumulating
                    # matmuls on fixed tiles, no cross-engine deps.
                    # mmprobe: 504 x [K=128 -> 512 cols]
                    # mmprobe64: same but K=64 (scores-like)
                    kk = 64 if upto == "mmprobe64" else 128
                    pp = ctx.enter_context(
                        tc.tile_pool(name="pp_ps", bufs=4, space="PSUM"))
                    for g in range(63):
                        ps = pp.tile([128, 512], F32, name="mp", tag="m")
                        for i in range(8):
                            nc.tensor.matmul(
                                ps, xt16[0][0:kk, 0:128],
                                xt16[1][0:kk, 0:512],
                                start=(i == 0), stop=(i == 7))
                    return

                qt = [persist.tile([128, SEQ], F16, name=f"qt{j}")
                      for j in range(3)]
                kt_ = [persist.tile([128, SEQ], F16, name=f"kt{j}")
                      for j in range(3)]
                vt = [persist.tile([128, H, DH + 1], F16, name=f"v{i}")
                      for i in range(NKT)]
                # pair-packed zT: rows 0..63 head 2j, 64..127 head 2j+1
                ztp = [persist.tile([128, SEQ], F16, name=f"ztp{j}")
                       for j in range(3)]

                s2_ps = ctx.enter_context(
                    tc.tile_pool(name="s2_ps", bufs=2, space="PSUM"))
                s_ps = ctx.enter_context(
                    tc.tile_pool(name="s_ps", bufs=2, space="PSUM"))
                z_ps = ctx.enter_context(
                    tc.tile_pool(name="z_ps", bufs=2, space="PSUM"))
                m_ps = s_ps
                pt_pool = ctx.enter_context(tc.tile_pool(name="pt_pool", bufs=10))
                r_pool = ctx.enter_context(tc.tile_pool(name="r_pool", bufs=3))
                rb_pool = ctx.enter_context(tc.tile_pool(name="rb_pool", bufs=3))
                zo_pool = ctx.enter_context(tc.tile_pool(name="zo_pool", bufs=3))
                zc_pool = ctx.enter_context(tc.tile_pool(name="zc_pool", bufs=3))
                o_sb_pool = ctx.enter_context(tc.tile_pool(name="o_sb", bufs=2))

                def proj_pair(j, with_v):
                    for dst, w, b_s in [(qt, "wq", bq_s), (kt_, "wk", bk_s)]:
                        for qc in range(NQC):
                            ps = m_ps.tile([128, 512], F32, name="ps", tag="s")
                            for c in range(MC):
                                nc.tensor.matmul(
                                    ps,
                                    w16[w][:, c, j * 128:(j + 1) * 128],
                                    xt16[c][:, qc * 512:(qc + 1) * 512],
                                    start=(c == 0), stop=(c == MC - 1))
                            if not skip_pmisc:
                                nc.vector.tensor_scalar(
                                    out=dst[j][:, qc * 512:(qc + 1) * 512],
                                    in0=ps, scalar1=b_s[:, j:j + 1],
                                    scalar2=None, op0=mybir.AluOpType.add)
                    if with_v:
                        for ktile in range(NKT):
                            ps = m_ps.tile([128, HD], F32, name="psv", tag="s")
                            for c in range(MC):
                                nc.tensor.matmul(
                                    ps,
                                    xt16[c][:, ktile * 128:(ktile + 1) * 128],
                                    w16["wv"][:, c, :],
                                    start=(c == 0), stop=(c == MC - 1))
                            if not skip_pmisc:
                                nc.vector.tensor_copy(
                                    vt[ktile][:, :, 0:DH],
                                    ps.rearrange("p (h d) -> p h d", h=H))
                                nc.vector.memset(
                                    vt[ktile][:, :, DH:DH + 1], 1.0)

                # timing probes: "attn_nox" removes exp (PV reads a dummy
                # SBUF tile) to measure pure-PE throughput; "attn_nopv"
                # removes PV+normalize to measure scores+exp only.
                # nox1/nox2/nox3: cumulative strip-downs from nox:
                #   nox1 = nox + no normalize chain
                #   nox2 = nox1 + no outproj copies/DMA
                #   nox3 = nox2 + no proj bias/V copies (pure PE stream)
                nox = upto in ("attn_nox", "nox1", "nox2", "nox3")
                nopv = upto == "attn_nopv"
                skip_norm = upto in ("nox1", "nox2", "nox3")
                skip_odve = upto in ("nox2", "nox3")
                skip_pmisc = upto == "nox3"
                if skip_norm:
                    for j in range(3):
                        nc.vector.memset(ztp[j], 0.5)
                if skip_pmisc:
                    for t in qt + kt_ + vt:
                        nc.vector.memset(t, 0.5)
                if nox:
                    p_dummy = consts.tile([128, 1024], F16)
                    nc.vector.memset(p_dummy, 0.001)

                def attn_head_qc(h, qc):
                    j, hp = h // 2, (h % 2) * 64
                    qc0 = qc * 512
                    z_t = z_ps.tile([65, 512], F32, name="z_t", tag="z")
                    strips = []

                    def emit_pv(ktile, p_t, base, cs):
                        if nopv:
                            return
                        src = p_dummy if nox else p_t
                        nc.tensor.matmul(
                            z_t[:, cs - qc0:512],
                            vt[ktile][:, h, :],
                            src[:, base + cs - qc0:base + 512],
                            start=(ktile == 0),
                            stop=(ktile == 4 * qc + 3))

                    def flush(n):
                        while len(strips) > n:
                            emit_pv(*strips.pop(0))

                    # paired full-width strips (non-diagonal)
                    for kp in range(2 * qc):
                        s_t = s2_ps.tile([128, 1024], F32, name="sp", tag="s2")
                        for idx in (0, 1):
                            ktile = 2 * kp + idx
                            k0 = ktile * 128
                            nc.tensor.matmul(
                                s_t[:, idx * 512:idx * 512 + 512],
                                kt_[j][hp:hp + 64, k0:k0 + 128],
                                qt[j][hp:hp + 64, qc0:qc0 + 512],
                                start=True, stop=True)
                        p_t = pt_pool.tile([128, 1024], F16, name="pp",
                                           tag="pt2", bufs=4)
                        if not nox:
                            nc.scalar.activation(
                                p_t, s_t, mybir.ActivationFunctionType.Exp)
                        strips.append((2 * kp, p_t, 0, qc0))
                        strips.append((2 * kp + 1, p_t, 512, qc0))
                        flush(2)
                    # diagonal strips
                    for ktile in range(4 * qc, 4 * qc + 4):
                        k0 = ktile * 128
                        cs = max(qc0, k0)
                        diag = cs == k0
                        s_t = s_ps.tile([128, 512], F32, name="s_t", tag="s")
                        nc.tensor.matmul(
                            s_t[:, cs - qc0:512],
                            kt_[j][hp:hp + 64, k0:k0 + 128],
                            qt[j][hp:hp + 64, cs:qc0 + 512],
                            start=True, stop=not diag)
                        if diag:
                            nc.tensor.matmul(
                                s_t[:, k0 - qc0:k0 - qc0 + 128],
                                ident, mneg, start=False, stop=True)
                        p_t = pt_pool.tile([128, 512], F16, name="p_t",
                                           tag="pt")
                        if not nox:
                            nc.scalar.activation(
                                p_t[:, cs - qc0:512], s_t[:, cs - qc0:512],
                                mybir.ActivationFunctionType.Exp)
                        strips.append((ktile, p_t, 0, cs))
                        flush(1)
                    flush(0)

                    if nopv or skip_norm:
                        return
                    # normalize -> pair-packed zT, split in two halves:
                    # A (now): copy z psum -> sbuf fp16 (frees the psum bank
                    #   fast), reciprocal of the sum row, and kick off a DMA
                    #   that replicates 1/Z across 64 partitions.
                    # B (returned; emitted one head later): fp16 SBUF multiply
                    #   + pair-pack, so the DVE never stalls on the DMA.
                    z16 = zc_pool.tile([65, 512], F16, name="z16", tag="zc")
                    with nc.allow_low_precision(
                            reason="z/Z in fp16, ~5e-4 rel; within budget"):
                        nc.vector.tensor_copy(z16, z_t[0:65, :])
                        r16 = r_pool.tile([1, 512], F16, name="r16", tag="r")
                        nc.vector.reciprocal(out=r16, in_=z16[64:65, :])
                    rb_t = rb_pool.tile([64, 512], F16, name="rb", tag="rb")
                    nc.sync.dma_start(
                        out=rb_t,
                        in_=r16.unsqueeze(1).to_broadcast([1, 64, 512]))

                    def finalize():
                        if hp == 0:
                            nc.vector.tensor_tensor(
                                out=ztp[j][0:64, qc0:qc0 + 512],
                                in0=z16[0:64, :], in1=rb_t,
                                op=mybir.AluOpType.mult)
                        else:
                            zo_t = zo_pool.tile([64, 512], F16, name="zo",
                                                tag="zo")
                            nc.vector.tensor_tensor(
                                out=zo_t, in0=z16[0:64, :], in1=rb_t,
                                op=mybir.AluOpType.mult)
                            nc.sync.dma_start(
                                out=ztp[j][64:128, qc0:qc0 + 512], in_=zo_t)
                    return finalize

                def outproj_qc(qc):
                    # 4 qtiles of this q-chunk -> one [128, 4, 768] sbuf tile
                    # -> single batched DMA to DRAM rows [qc*512, qc*512+512)
                    o_s = o_sb_pool.tile([128, NQC, DM], F16, name="o_s",
                                         tag="os")
                    for t in range(4):
                        qtile = 4 * qc + t
                        for n0, w in [(0, 512), (512, 256)]:
                            if (qtile * 2 + (n0 > 0)) % 2 == 0:
                                o_t = s2_ps.tile([128, 512], F32, name="o_t",
                                                 tag="s2", bufs=2)
                            else:
                                o_t = s_ps.tile([128, 512], F32, name="o_t2",
                                                tag="s")
                            for j in range(3):
                                nc.tensor.matmul(
                                    o_t[:, 0:w],
                                    ztp[j][:, qtile * 128:(qtile + 1) * 128],
                                    wo16[j][:, n0:n0 + w],
                                    start=(j == 0), stop=(j == 2))
                            if not skip_odve:
                                nc.vector.tensor_copy(o_s[:, t, n0:n0 + w],
                                                      o_t[:, 0:w])
                    if not skip_odve:
                        nc.sync.dma_start(
                            out=o_d.ap()[qc * 512:(qc + 1) * 512, :].rearrange(
                                "(t p) m -> p t m", p=128),
                            in_=o_s)

                proj_pair(0, with_v=True)
                if upto == "proj":
                    proj_pair(1, with_v=False)
                    proj_pair(2, with_v=False)
                    return

                pending_fin = None
                for qc in range(NQC):
                    for h in range(H):
                        if qc == 0 and h in (2, 4):
                            proj_pair(h // 2, with_v=False)
                        fin = attn_head_qc(h, qc)
                        if pending_fin is not None:
                            pending_fin()
                        pending_fin = fin
                    if upto not in ("attn", "attn_nopv"):
                        if pending_fin is not None:
                            pending_fin()
                            pending_fin = None
                        outproj_qc(qc)
                if upto.startswith("nox"):
                    return

        if reps == 1:
            body()
        else:
            with tc.For_i(0, reps, 1) as _iv:
                body(_iv)

    nc.compile()
    _BUILD_CACHE[key] = nc
    return nc


def make_in_maps(normalized_resid_pre, W_Q, W_K, W_V, W_O, b_Q, b_K, b_V, b_O):
    scale = np.float32(1.0 / np.sqrt(DH))
    in_maps = []
    for core in range(8):
        b, h0 = core // 2, (core % 2) * H
        hs = slice(h0, h0 + H)
        in_maps.append({
            "xt": np.ascontiguousarray(
                normalized_resid_pre[b].T).astype(np.float16),
            "wq": (np.ascontiguousarray(
                W_Q[hs].transpose(1, 0, 2).reshape(MC, 128, HD)
                .transpose(1, 0, 2)) * scale).astype(np.float16),
            "wk": np.ascontiguousarray(
                W_K[hs].transpose(1, 0, 2).reshape(MC, 128, HD)
                .transpose(1, 0, 2)).astype(np.float16),
            "wv": np.ascontiguousarray(
                W_V[hs].transpose(1, 0, 2).reshape(MC, 128, HD)
                .transpose(1, 0, 2)).astype(np.float16),
            "wo": np.ascontiguousarray(
                W_O[hs].reshape(3, 128, DM)).astype(np.float16),
            "bq": np.ascontiguousarray(
                (np.asarray(b_Q)[hs].reshape(HD, 1).reshape(HD // 128, 128).T
                 * scale)).astype(np.float32),
            "bk": np.ascontiguousarray(
                np.asarray(b_K)[hs].reshape(HD // 128, 128).T).astype(
                np.float32),
        })
    return in_maps


def assemble(results, b_V, b_O, W_O):
    bv_wo = np.einsum("hd,hdm->m", b_V.astype(np.float64),
                      W_O.astype(np.float64)).astype(np.float32)
    out = np.empty((BATCH, SEQ, DM), dtype=np.float32)
    for b in range(BATCH):
        out[b] = (results[2 * b]["out"].astype(np.float32)
                  + results[2 * b + 1]["out"].astype(np.float32)
                  + b_O + bv_wo)
    return out


def kernel(normalized_resid_pre, W_Q, W_K, W_V, W_O, b_Q, b_K, b_V, b_O):
    nc = build(reps=1)
    in_maps = make_in_maps(normalized_resid_pre, W_Q, W_K, W_V, W_O,
                           b_Q, b_K, b_V, b_O)
    last_err = None
    for _attempt in range(3):
        try:
            res = run_bass_kernel_spmd(nc, in_maps, core_ids=list(range(8)))
            return assemble(res.results, b_V, b_O, W_O)
        except Exception as e:  # transient NRT/axon hiccups observed
            last_err = e
    raise last_err
